# revision 3
# baseline (speedup 1.0000x reference)
"""Multi-head attention (B=1, H=64, S=362, D=506) with softmax + dropout(p=0.1,
train, jax key 42) on 8 trn2 NeuronCores, head-parallel (8 heads per core).

Host side: pre-transposes Q/K per head to [D, S] (so the device needs no
Q/K transposes), folds the 1/sqrt(D) scale into Q, and precomputes the
jax threefry dropout keep-mask as a {0, 1/0.9} f32 multiplier.

Device side per head: QK^T -> PSUM, DVE adds attn_mask in place, ACT exp
with accum_out (softmax denominator for free), DVE reciprocal, one fused
scalar_tensor_tensor for (exp * 1/sum) * dropmask, PE-transpose of the
attention matrix, AV matmul against natural-layout V, natural [S, D] out.
"""

import math
from contextlib import ExitStack

import numpy as np

import concourse.bass as bass
import concourse.mybir as mybir
import concourse.tile as tile
from concourse import bacc
from concourse.bass_utils import run_bass_kernel_spmd
from concourse.masks import make_identity

B, H, S, D = 1, 64, 362, 506
N_CORES = 8
HPC = H // N_CORES  # heads per core
DROP_P = 0.1

F32 = mybir.dt.float32

# Matmul dtype mode: "f32" (exact, 4 cyc/row), "f32r" (1 cyc/row, reduced
# precision multiply), set before first kernel() call.
MM_MODE = "f32r"
TRACE = False
LAST_RESULTS = None

_Q_CHUNKS = [(0, 128), (128, 128), (256, 106)]          # S = 362
_K_CHUNKS = _Q_CHUNKS
_D_CHUNKS = [(0, 128), (128, 128), (256, 128), (384, 122)]  # D = 506

_CACHE = {}


def _mm_dt():
    return {"f32": mybir.dt.float32, "f32r": mybir.dt.float32r}[MM_MODE]


def _mm(ap):
    """View a f32 AP with the matmul dtype."""
    dt = _mm_dt()
    return ap if dt == F32 else ap.bitcast(dt)


def _build_nc():
    nc = bacc.Bacc(
        "TRN2",
        target_bir_lowering=False,
        debug=False,
        num_devices=N_CORES,
    )
    qT = nc.dram_tensor("qT", [HPC, D, S], F32, kind="ExternalInput").ap()
    kT = nc.dram_tensor("kT", [HPC, D, S], F32, kind="ExternalInput").ap()
    v = nc.dram_tensor("v", [HPC, S, D], F32, kind="ExternalInput").ap()
    msk = nc.dram_tensor("msk", [S, S], F32, kind="ExternalInput").ap()
    keep = nc.dram_tensor("keep", [HPC, S, S], F32, kind="ExternalInput").ap()
    out = nc.dram_tensor("out", [HPC, S, D], F32, kind="ExternalOutput").ap()

    with tile.TileContext(nc) as tc, ExitStack() as ctx:
        const = ctx.enter_context(tc.tile_pool(name="const", bufs=1))
        io = ctx.enter_context(tc.tile_pool(name="io", bufs=2))
        work = ctx.enter_context(tc.tile_pool(name="work", bufs=2))
        small = ctx.enter_context(tc.tile_pool(name="small", bufs=4))
        ps_s = ctx.enter_context(tc.tile_pool(name="ps_s", bufs=3, space="PSUM"))
        ps_t = ctx.enter_context(tc.tile_pool(name="ps_t", bufs=2, space="PSUM"))
        ps_o = ctx.enter_context(tc.tile_pool(name="ps_o", bufs=3, space="PSUM"))

        ident = const.tile([128, 128], F32, tag="ident")
        make_identity(nc, ident[:])

        msk_sb = []
        for qc, (qs, qsz) in enumerate(_Q_CHUNKS):
            m = const.tile([qsz, S], F32, tag=f"msk{qc}")
            nc.sync.dma_start(m[:], msk[qs : qs + qsz, :])
            msk_sb.append(m)

        for h in range(HPC):
            # ---- loads ----
            qt, kt = [], []
            for ds, dsz in _D_CHUNKS:
                t = io.tile([dsz, S], F32, tag="qt", bufs=10)
                nc.sync.dma_start(t[:], qT[h, ds : ds + dsz, :])
                qt.append(t)
                t = io.tile([dsz, S], F32, tag="kt", bufs=10)
                nc.sync.dma_start(t[:], kT[h, ds : ds + dsz, :])
                kt.append(t)
            vv = []
            for ks, ksz in _K_CHUNKS:
                t = io.tile([ksz, D], F32, tag="v", bufs=8)
                nc.sync.dma_start(t[:], v[h, ks : ks + ksz, :])
                vv.append(t)
            kp = []
            for qs, qsz in _Q_CHUNKS:
                t = io.tile([qsz, S], F32, tag="keep", bufs=8)
                nc.sync.dma_start(t[:], keep[h, qs : qs + qsz, :])
                kp.append(t)

            # ---- scores + softmax + dropout ----
            att = []
            for qc, (qs, qsz) in enumerate(_Q_CHUNKS):
                ps = ps_s.tile([qsz, S], F32, tag="ps")
                for dc in range(len(_D_CHUNKS)):
                    nc.tensor.matmul(
                        ps[:],
                        _mm(qt[dc][:, qs : qs + qsz]),
                        _mm(kt[dc][:]),
                        start=(dc == 0),
                        stop=(dc == len(_D_CHUNKS) - 1),
                    )
                # scores += mask (in place in PSUM)
                nc.vector.tensor_tensor(
                    ps[:], ps[:], msk_sb[qc][:], op=mybir.AluOpType.add
                )
                e = work.tile([qsz, S], F32, tag="e", bufs=4)
                ssum = small.tile([qsz, 1], F32, tag="ssum")
                nc.scalar.activation(
                    e[:], ps[:], mybir.ActivationFunctionType.Exp, accum_out=ssum[:]
                )
                r = small.tile([qsz, 1], F32, tag="r")
                nc.vector.reciprocal(r[:], ssum[:])
                a = work.tile([qsz, S], F32, tag="a", bufs=6)
                nc.vector.scalar_tensor_tensor(
                    a[:],
                    in0=e[:],
                    scalar=r[:],
                    in1=kp[qc][:],
                    op0=mybir.AluOpType.mult,
                    op1=mybir.AluOpType.mult,
                )
                att.append(a)

            # ---- transpose attn: aT[kc][:, q] = att[qc][:, k].T ----
            aT = []
            for kc, (ks, ksz) in enumerate(_K_CHUNKS):
                pt = ps_t.tile([ksz, S], F32, tag="pt")
                for qc, (qs, qsz) in enumerate(_Q_CHUNKS):
                    nc.tensor.transpose(
                        pt[:, qs : qs + qsz],
                        att[qc][:, ks : ks + ksz],
                        ident[:qsz, :qsz],
                    )
                t = work.tile([ksz, S], F32, tag="aT", bufs=6)
                nc.scalar.copy(t[:], pt[:])
                aT.append(t)

            # ---- AV + store ----
            for qc, (qs, qsz) in enumerate(_Q_CHUNKS):
                po = ps_o.tile([qsz, D], F32, tag="po")
                for kc, (ks, ksz) in enumerate(_K_CHUNKS):
                    nc.tensor.matmul(
                        po[:],
                        _mm(aT[kc][:, qs : qs + qsz]),
                        _mm(vv[kc][:]),
                        start=(kc == 0),
                        stop=(kc == len(_K_CHUNKS) - 1),
                    )
                o = work.tile([qsz, D], F32, tag="o", bufs=3)
                nc.vector.tensor_copy(o[:], po[:])
                nc.sync.dma_start(out[h, qs : qs + qsz, :], o[:])

    nc.finalize()
    return nc


def _get_nc():
    key = MM_MODE
    if key not in _CACHE:
        _CACHE[key] = _build_nc()
    return _CACHE[key]


def _keep_mask():
    """Dropout keep mask, identical bits to the reference (threefry is
    backend-deterministic); computed on the CPU backend."""
    import jax

    cpu = jax.devices("cpu")[0]
    with jax.default_device(cpu):
        k = jax.random.bernoulli(jax.random.key(42), 1.0 - DROP_P, (B, H, S, S))
        return np.asarray(k)


def kernel(query, key, value, attn_mask):
    global LAST_RESULTS
    q = np.asarray(query, dtype=np.float32)[0]  # [H, S, D]
    k = np.asarray(key, dtype=np.float32)[0]
    v = np.asarray(value, dtype=np.float32)[0]
    msk = np.ascontiguousarray(np.asarray(attn_mask, dtype=np.float32)[0, 0])

    scale = 1.0 / math.sqrt(D)
    qT = np.ascontiguousarray(np.transpose(q, (0, 2, 1)) * scale)  # [H, D, S]
    kT = np.ascontiguousarray(np.transpose(k, (0, 2, 1)))  # [H, D, S]
    keepf = _keep_mask()[0].astype(np.float32) * np.float32(1.0 / (1.0 - DROP_P))

    nc = _get_nc()
    in_maps = []
    for c in range(N_CORES):
        sl = slice(c * HPC, (c + 1) * HPC)
        in_maps.append(
            {
                "qT": qT[sl],
                "kT": kT[sl],
                "v": np.ascontiguousarray(v[sl]),
                "msk": msk,
                "keep": np.ascontiguousarray(keepf[sl]),
            }
        )

    res = run_bass_kernel_spmd(nc, in_maps, list(range(N_CORES)), trace=TRACE)
    LAST_RESULTS = res
    out = np.concatenate([res.results[c]["out"] for c in range(N_CORES)], axis=0)
    return out.reshape(B, H, S, D).astype(np.float32)


# revision 7
# speedup vs baseline: 1.0261x; 1.0261x over previous
"""Multi-head attention (B=1, H=64, S=362, D=506) with softmax + dropout(p=0.1,
train, jax key 42) on 8 trn2 NeuronCores, head-parallel (8 heads per core).

Host side: pre-transposes Q/K per head to [D, S] (so the device needs no
Q/K transposes), folds the 1/sqrt(D) scale into Q, and precomputes the
jax threefry dropout keep-mask as a {0, 1/0.9} f32 multiplier.

Device side per head: QK^T -> PSUM, DVE adds attn_mask in place, ACT exp
with accum_out (softmax denominator for free), DVE reciprocal, one fused
scalar_tensor_tensor for (exp * 1/sum) * dropmask, PE-transpose of the
attention matrix, AV matmul against natural-layout V, natural [S, D] out.
"""

import math
from contextlib import ExitStack

import numpy as np

import concourse.bass as bass
import concourse.mybir as mybir
import concourse.tile as tile
from concourse import bacc
from concourse.bass_utils import run_bass_kernel_spmd
from concourse.masks import make_identity

B, H, S, D = 1, 64, 362, 506
N_CORES = 8
HPC = H // N_CORES  # heads per core
DROP_P = 0.1

F32 = mybir.dt.float32

# Matmul dtype mode: "f32" (exact, 4 cyc/row), "f32r" (1 cyc/row, reduced
# precision multiply), set before first kernel() call.
MM_MODE = "f32r"
TRACE = False
LAST_RESULTS = None

_Q_CHUNKS = [(0, 128), (128, 128), (256, 106)]          # S = 362
_K_CHUNKS = _Q_CHUNKS
_D_CHUNKS = [(0, 128), (128, 128), (256, 128), (384, 122)]  # D = 506

_CACHE = {}


def _mm_dt():
    return {"f32": mybir.dt.float32, "f32r": mybir.dt.float32r}[MM_MODE]


def _build_nc():
    MM = _mm_dt()
    nc = bacc.Bacc(
        "TRN2",
        target_bir_lowering=False,
        debug=False,
        num_devices=N_CORES,
    )
    qT = nc.dram_tensor("qT", [HPC, D, S], MM, kind="ExternalInput").ap()
    kT = nc.dram_tensor("kT", [HPC, D, S], MM, kind="ExternalInput").ap()
    v = nc.dram_tensor("v", [HPC, S, D], MM, kind="ExternalInput").ap()
    msk = nc.dram_tensor("msk", [S, S], F32, kind="ExternalInput").ap()
    keep = nc.dram_tensor("keep", [HPC, S, S], F32, kind="ExternalInput").ap()
    out = nc.dram_tensor("out", [HPC, S, D], F32, kind="ExternalOutput").ap()

    with tile.TileContext(nc) as tc, ExitStack() as ctx:
        const = ctx.enter_context(tc.tile_pool(name="const", bufs=1))
        io = ctx.enter_context(tc.tile_pool(name="io", bufs=2))
        work = ctx.enter_context(tc.tile_pool(name="work", bufs=2))
        small = ctx.enter_context(tc.tile_pool(name="small", bufs=4))
        ps_s = ctx.enter_context(tc.tile_pool(name="ps_s", bufs=3, space="PSUM"))
        ps_t = ctx.enter_context(tc.tile_pool(name="ps_t", bufs=2, space="PSUM"))
        ps_o = ctx.enter_context(tc.tile_pool(name="ps_o", bufs=3, space="PSUM"))

        ident_f32 = const.tile([128, 128], F32, tag="ident_f32")
        make_identity(nc, ident_f32[:])
        if MM == F32:
            ident = ident_f32
        else:
            # fp32r matmul operands must come from a rounding producer
            ident = const.tile([128, 128], MM, tag="ident")
            nc.scalar.copy(ident[:], ident_f32[:])

        msk_sb = []
        for qc, (qs, qsz) in enumerate(_Q_CHUNKS):
            m = const.tile([qsz, S], F32, tag=f"msk{qc}")
            nc.sync.dma_start(m[:], msk[qs : qs + qsz, :])
            msk_sb.append(m)

        for h in range(HPC):
            # ---- loads ----
            qt, kt = [], []
            for ds, dsz in _D_CHUNKS:
                t = io.tile([dsz, S], MM, tag="qt", bufs=10)
                nc.sync.dma_start(t[:], qT[h, ds : ds + dsz, :])
                qt.append(t)
                t = io.tile([dsz, S], MM, tag="kt", bufs=10)
                nc.sync.dma_start(t[:], kT[h, ds : ds + dsz, :])
                kt.append(t)
            vv = []
            for ks, ksz in _K_CHUNKS:
                t = io.tile([ksz, D], MM, tag="v", bufs=8)
                nc.sync.dma_start(t[:], v[h, ks : ks + ksz, :])
                vv.append(t)
            kp = []
            for qs, qsz in _Q_CHUNKS:
                t = io.tile([qsz, S], F32, tag="keep", bufs=8)
                nc.sync.dma_start(t[:], keep[h, qs : qs + qsz, :])
                kp.append(t)

            # ---- scores + softmax + dropout ----
            att = []
            for qc, (qs, qsz) in enumerate(_Q_CHUNKS):
                ps = ps_s.tile([qsz, S], F32, tag="ps")
                for dc in range(len(_D_CHUNKS)):
                    nc.tensor.matmul(
                        ps[:],
                        qt[dc][:, qs : qs + qsz],
                        kt[dc][:],
                        start=(dc == 0),
                        stop=(dc == len(_D_CHUNKS) - 1),
                    )
                # scores += mask (in place in PSUM)
                nc.vector.tensor_tensor(
                    ps[:], ps[:], msk_sb[qc][:], op=mybir.AluOpType.add
                )
                e = work.tile([qsz, S], F32, tag="e", bufs=4)
                ssum = small.tile([qsz, 1], F32, tag="ssum")
                nc.scalar.activation(
                    e[:], ps[:], mybir.ActivationFunctionType.Exp, accum_out=ssum[:]
                )
                r = small.tile([qsz, 1], F32, tag="r")
                nc.vector.reciprocal(r[:], ssum[:])
                a = work.tile([qsz, S], MM, tag="a", bufs=6)
                nc.vector.scalar_tensor_tensor(
                    a[:],
                    in0=e[:],
                    scalar=r[:],
                    in1=kp[qc][:],
                    op0=mybir.AluOpType.mult,
                    op1=mybir.AluOpType.mult,
                )
                att.append(a)

            # ---- transpose attn: aT[kc][:, q] = att[qc][:, k].T ----
            aT = []
            for kc, (ks, ksz) in enumerate(_K_CHUNKS):
                pt = ps_t.tile([ksz, S], MM, tag="pt")
                for qc, (qs, qsz) in enumerate(_Q_CHUNKS):
                    nc.tensor.transpose(
                        pt[:, qs : qs + qsz],
                        att[qc][:, ks : ks + ksz],
                        ident[:qsz, :qsz],
                    )
                t = work.tile([ksz, S], MM, tag="aT", bufs=6)
                nc.scalar.copy(t[:], pt[:])
                aT.append(t)

            # ---- AV + store ----
            for qc, (qs, qsz) in enumerate(_Q_CHUNKS):
                po = ps_o.tile([qsz, D], F32, tag="po")
                for kc, (ks, ksz) in enumerate(_K_CHUNKS):
                    nc.tensor.matmul(
                        po[:],
                        aT[kc][:, qs : qs + qsz],
                        vv[kc][:],
                        start=(kc == 0),
                        stop=(kc == len(_K_CHUNKS) - 1),
                    )
                o = work.tile([qsz, D], F32, tag="o", bufs=3)
                nc.vector.tensor_copy(o[:], po[:])
                nc.sync.dma_start(out[h, qs : qs + qsz, :], o[:])

    nc.finalize()
    return nc


def _get_nc():
    key = MM_MODE
    if key not in _CACHE:
        _CACHE[key] = _build_nc()
    return _CACHE[key]


def _keep_mask():
    """Dropout keep mask, identical bits to the reference (threefry is
    backend-deterministic); computed on the CPU backend."""
    import jax

    cpu = jax.devices("cpu")[0]
    with jax.default_device(cpu):
        k = jax.random.bernoulli(jax.random.key(42), 1.0 - DROP_P, (B, H, S, S))
        return np.asarray(k)


def kernel(query, key, value, attn_mask):
    global LAST_RESULTS
    q = np.asarray(query, dtype=np.float32)[0]  # [H, S, D]
    k = np.asarray(key, dtype=np.float32)[0]
    v = np.asarray(value, dtype=np.float32)[0]
    msk = np.ascontiguousarray(np.asarray(attn_mask, dtype=np.float32)[0, 0])

    scale = 1.0 / math.sqrt(D)
    qT = np.ascontiguousarray(np.transpose(q, (0, 2, 1)) * scale)  # [H, D, S]
    kT = np.ascontiguousarray(np.transpose(k, (0, 2, 1)))  # [H, D, S]
    keepf = _keep_mask()[0].astype(np.float32) * np.float32(1.0 / (1.0 - DROP_P))

    nc = _get_nc()
    in_maps = []
    for c in range(N_CORES):
        sl = slice(c * HPC, (c + 1) * HPC)
        in_maps.append(
            {
                "qT": qT[sl],
                "kT": kT[sl],
                "v": np.ascontiguousarray(v[sl]),
                "msk": msk,
                "keep": np.ascontiguousarray(keepf[sl]),
            }
        )

    res = run_bass_kernel_spmd(nc, in_maps, list(range(N_CORES)), trace=TRACE)
    LAST_RESULTS = res
    out = np.concatenate([res.results[c]["out"] for c in range(N_CORES)], axis=0)
    return out.reshape(B, H, S, D).astype(np.float32)


# revision 8
# speedup vs baseline: 2.4529x; 2.3906x over previous
"""Multi-head attention (B=1, H=64, S=362, D=506) with softmax + dropout(p=0.1,
train, jax key 42) on 8 trn2 NeuronCores, head-parallel (8 heads per core).

Host side: pre-transposes Q/K per head to [D, S] (so the device needs no
Q/K transposes), folds the 1/sqrt(D) scale into Q and the 1/(1-p) dropout
scale into V, pads D->512 and S->384 so every DMA is one large contiguous
transfer per tensor per head (descriptor-efficiency), and precomputes the
jax threefry dropout keep-mask as a {0,1} uint8.

Device side per head: QK^T -> PSUM, DVE adds attn_mask in place, ACT exp
with accum_out (softmax denominator for free), DVE reciprocal, one fused
scalar_tensor_tensor for (exp * 1/sum) * keepmask, PE-transpose of the
attention matrix, AV matmul against natural-layout V, natural [S, D] out.
DMA loads are split across both HWDGE rings (sync + scalar); stores go
through SWDGE (gpsimd) to keep all rings busy.
"""

import math
from contextlib import ExitStack

import numpy as np

import concourse.bass as bass
import concourse.mybir as mybir
import concourse.tile as tile
from concourse import bacc
from concourse.bass_utils import run_bass_kernel_spmd
from concourse.masks import make_identity

B, H, S, D = 1, 64, 362, 506
SP = 384  # S padded to 3*128
DP = 512  # D padded to 4*128
N_CORES = 8
HPC = H // N_CORES  # heads per core
DROP_P = 0.1

F32 = mybir.dt.float32
U8 = mybir.dt.uint8

# Matmul dtype mode: "f32" (exact, 4 cyc/row), "f32r" (1 cyc/row, reduced
# precision multiply). Set before first kernel() call.
MM_MODE = "f32r"
TRACE = False
LAST_RESULTS = None

_Q_CHUNKS = [(0, 128), (128, 128), (256, 106)]  # S = 362 (+pad to 384)
_K_CHUNKS = _Q_CHUNKS

_CACHE = {}


def _mm_dt():
    return {"f32": mybir.dt.float32, "f32r": mybir.dt.float32r}[MM_MODE]


def _build_nc():
    MM = _mm_dt()
    nc = bacc.Bacc(
        "TRN2",
        target_bir_lowering=False,
        debug=False,
        num_devices=N_CORES,
    )
    qT = nc.dram_tensor("qT", [HPC, DP, S], MM, kind="ExternalInput").ap()
    kT = nc.dram_tensor("kT", [HPC, DP, S], MM, kind="ExternalInput").ap()
    v = nc.dram_tensor("v", [HPC, SP, D], MM, kind="ExternalInput").ap()
    msk = nc.dram_tensor("msk", [SP, S], F32, kind="ExternalInput").ap()
    keep = nc.dram_tensor("keep", [HPC, SP, S], U8, kind="ExternalInput").ap()
    out = nc.dram_tensor("out", [HPC, SP, D], F32, kind="ExternalOutput").ap()

    with tile.TileContext(nc) as tc, ExitStack() as ctx:
        const = ctx.enter_context(tc.tile_pool(name="const", bufs=1))
        io = ctx.enter_context(tc.tile_pool(name="io", bufs=2))
        work = ctx.enter_context(tc.tile_pool(name="work", bufs=2))
        small = ctx.enter_context(tc.tile_pool(name="small", bufs=4))
        ps_s = ctx.enter_context(tc.tile_pool(name="ps_s", bufs=3, space="PSUM"))
        ps_t = ctx.enter_context(tc.tile_pool(name="ps_t", bufs=2, space="PSUM"))
        ps_o = ctx.enter_context(tc.tile_pool(name="ps_o", bufs=3, space="PSUM"))

        ident_f32 = const.tile([128, 128], F32, tag="ident_f32")
        make_identity(nc, ident_f32[:])
        if MM == F32:
            ident = ident_f32
        else:
            # fp32r matmul operands must come from a rounding producer
            ident = const.tile([128, 128], MM, tag="ident")
            nc.scalar.copy(ident[:], ident_f32[:])

        # attn mask, one batched load: [q-part, qc, k]
        msk_sb = const.tile([128, 3, S], F32, tag="msk")
        nc.sync.dma_start(msk_sb[:], msk.rearrange("(c p) s -> p c s", p=128))

        for h in range(HPC):
            # ---- loads (one large DMA per tensor per head) ----
            qt = io.tile([128, 4, S], MM, tag="qt", bufs=3)
            nc.sync.dma_start(qt[:], qT[h].rearrange("(c p) s -> p c s", p=128))
            kt = io.tile([128, 4, S], MM, tag="kt", bufs=3)
            nc.scalar.dma_start(kt[:], kT[h].rearrange("(c p) s -> p c s", p=128))
            vv = io.tile([128, 3, D], MM, tag="v", bufs=3)
            nc.sync.dma_start(vv[:], v[h].rearrange("(c p) d -> p c d", p=128))
            kp = io.tile([128, 3, S], U8, tag="keep", bufs=3)
            nc.scalar.dma_start(kp[:], keep[h].rearrange("(c p) s -> p c s", p=128))

            # ---- scores + softmax + dropout ----
            att = []
            for qc, (qs, qsz) in enumerate(_Q_CHUNKS):
                ps = ps_s.tile([qsz, S], F32, tag="ps")
                for dc in range(4):
                    nc.tensor.matmul(
                        ps[:],
                        qt[:, dc, qs : qs + qsz],
                        kt[:, dc, :],
                        start=(dc == 0),
                        stop=(dc == 3),
                    )
                # scores += mask (in place in PSUM)
                nc.vector.tensor_tensor(
                    ps[:], ps[:], msk_sb[:qsz, qc, :], op=mybir.AluOpType.add
                )
                e = work.tile([qsz, S], F32, tag="e", bufs=4)
                ssum = small.tile([qsz, 1], F32, tag="ssum")
                nc.scalar.activation(
                    e[:], ps[:], mybir.ActivationFunctionType.Exp, accum_out=ssum[:]
                )
                r = small.tile([qsz, 1], F32, tag="r")
                nc.vector.reciprocal(r[:], ssum[:])
                a = work.tile([qsz, S], MM, tag="a", bufs=6)
                nc.vector.scalar_tensor_tensor(
                    a[:],
                    in0=e[:],
                    scalar=r[:],
                    in1=kp[:qsz, qc, :],
                    op0=mybir.AluOpType.mult,
                    op1=mybir.AluOpType.mult,
                )
                att.append(a)

            # ---- transpose attn: aT[kc][:, q] = att[qc][:, k].T ----
            aT = []
            for kc, (ks, ksz) in enumerate(_K_CHUNKS):
                pt = ps_t.tile([ksz, S], MM, tag="pt")
                for qc, (qs, qsz) in enumerate(_Q_CHUNKS):
                    nc.tensor.transpose(
                        pt[:, qs : qs + qsz],
                        att[qc][:, ks : ks + ksz],
                        ident[:qsz, :qsz],
                    )
                t = work.tile([ksz, S], MM, tag="aT", bufs=6)
                nc.scalar.copy(t[:], pt[:])
                aT.append(t)

            # ---- AV + store (one batched store per head) ----
            o = work.tile([128, 3, D], F32, tag="o", bufs=2)
            for qc, (qs, qsz) in enumerate(_Q_CHUNKS):
                po = ps_o.tile([qsz, D], F32, tag="po")
                for kc, (ks, ksz) in enumerate(_K_CHUNKS):
                    nc.tensor.matmul(
                        po[:],
                        aT[kc][:, qs : qs + qsz],
                        vv[:ksz, kc, :],
                        start=(kc == 0),
                        stop=(kc == len(_K_CHUNKS) - 1),
                    )
                nc.vector.tensor_copy(o[:qsz, qc, :], po[:])
            nc.gpsimd.dma_start(out[h].rearrange("(c p) d -> p c d", p=128), o[:])

    nc.finalize()
    return nc


def _get_nc():
    key = MM_MODE
    if key not in _CACHE:
        _CACHE[key] = _build_nc()
    return _CACHE[key]


def _keep_mask():
    """Dropout keep mask, identical bits to the reference (threefry is
    backend-deterministic); computed on the CPU backend."""
    import jax

    cpu = jax.devices("cpu")[0]
    with jax.default_device(cpu):
        k = jax.random.bernoulli(jax.random.key(42), 1.0 - DROP_P, (B, H, S, S))
        return np.asarray(k)


def kernel(query, key, value, attn_mask):
    global LAST_RESULTS
    q = np.asarray(query, dtype=np.float32)[0]  # [H, S, D]
    k = np.asarray(key, dtype=np.float32)[0]
    v = np.asarray(value, dtype=np.float32)[0]
    msk = np.zeros((SP, S), dtype=np.float32)
    msk[:S] = np.asarray(attn_mask, dtype=np.float32)[0, 0]

    scale = 1.0 / math.sqrt(D)
    qT = np.zeros((H, DP, S), dtype=np.float32)
    qT[:, :D] = np.transpose(q, (0, 2, 1)) * scale
    kT = np.zeros((H, DP, S), dtype=np.float32)
    kT[:, :D] = np.transpose(k, (0, 2, 1))
    vp = np.zeros((H, SP, D), dtype=np.float32)
    vp[:, :S] = v * np.float32(1.0 / (1.0 - DROP_P))
    keepu = np.zeros((H, SP, S), dtype=np.uint8)
    keepu[:, :S] = _keep_mask()[0]

    nc = _get_nc()
    in_maps = []
    for c in range(N_CORES):
        sl = slice(c * HPC, (c + 1) * HPC)
        in_maps.append(
            {
                "qT": np.ascontiguousarray(qT[sl]),
                "kT": np.ascontiguousarray(kT[sl]),
                "v": np.ascontiguousarray(vp[sl]),
                "msk": msk,
                "keep": np.ascontiguousarray(keepu[sl]),
            }
        )

    res = run_bass_kernel_spmd(nc, in_maps, list(range(N_CORES)), trace=TRACE)
    LAST_RESULTS = res
    out = np.concatenate(
        [res.results[c]["out"][:, :S, :] for c in range(N_CORES)], axis=0
    )
    return out.reshape(B, H, S, D).astype(np.float32)
